# revision 3
# baseline (speedup 1.0000x reference)
"""Trainium2 Bass kernel for CausalGraphNetwork — v2.

Computes, for x = step_sequence [B=2, N=512, H=256]:
    h  = relu(x @ W_gc1.T + b_gc1)
    f  = relu(h @ W_gc2.T + b_gc2)
    a  = f @ Wa.T + b_ep1    (Wa = W_ep1[:, :H])
    c  = f @ Wb.T            (Wb = W_ep1[:, H:])
    e[b,i,j,:] = relu(a[b,i,:] + c[b,j,:])
    scores = sigmoid(e @ w_ep2 + b_ep2) * strict_lower_mask

Strategy (8 NeuronCores, SPMD single program):
  - Core d owns batch d//4, rows i = 4k + (d%4), k = 0..127 (interleaved
    so causal work is identical on every core).
  - Row k's score row is accumulated at one PSUM partition of ONE score
    bank via a sliding one-hot lhsT: lhsT = wsl[:, 31-m:63-m] puts w at
    output row m of the 32-row column-tile quadrant (tile_position
    (0,32u)); all other rows accumulate +0.  128 rows -> one [128,512]
    bank, drained by 4 quadrant sigmoids + 4 contiguous DMAs.
  - e-gen (e = relu(c_j + a_k), per-partition bias) is split across
    DVE/Pool (tensor_scalar from SBUF c) and ACT (activation from a
    PSUM-resident copy of c, fused bias+relu), greedy-balanced.
  - Row processing order: k = 0..23 ascending (small causal extents,
    only c[:, :256] needed) then k = 127..24 descending, so the tail
    rows are cheap and quadrant drains overlap remaining compute.
  - Causality: row k computes jbx(k) = round8(4k+4) columns; the host
    applies the exact strict-lower mask after gathering.
"""

import ml_dtypes
import numpy as np

import bass_rust
import concourse.bass as bass
import concourse.mybir as mybir
import concourse.tile as tile
from concourse.bass_utils import run_bass_kernel_spmd
from concourse.vector_clock import ScopedClock

B, N, H = 2, 512, 256
NCORES = 8
R = 128  # rows per core
F32 = mybir.dt.float32
BF16 = mybir.dt.bfloat16
AF = mybir.ActivationFunctionType
ALU = mybir.AluOpType


def jbx(k: int) -> int:
    """Per-row compute extent (even, = 4k+4, capped at N)."""
    return min(N, 4 * k + 4)


# processing order: position -> row k
ORDER = list(range(24)) + list(range(127, 23, -1))
assert sorted(ORDER) == list(range(R))


def qmax(u: int) -> int:
    """Max compute extent among rows at positions 32u..32u+31."""
    return max(jbx(ORDER[p]) for p in range(32 * u, 32 * u + 32))


# ---- static engine assignment for the 256 e-gen chunk ops ----
def _mk_assignment():
    def dve_cost(jb):
        return 0.52 * jb + 30.0

    def pool_cost(jb):
        return 0.834 * jb + 40.0

    def act_cost(jb):
        return 0.833 * jb + 460.0

    loads = {"vector": 800.0, "gpsimd": 0.0, "scalar": 1800.0}
    costs = {"vector": dve_cost, "gpsimd": pool_cost, "scalar": act_cost}
    assign = []
    for pos in range(R):
        jb = jbx(ORDER[pos])
        row = []
        for c in range(2):
            opts = list(loads)
            if pos % 32 in (30, 31, 0, 1):  # keep ACT free around sigmoids
                opts = [o for o in opts if o != "scalar"]
            e = min(opts, key=lambda n: loads[n] + costs[n](jb))
            loads[e] += costs[e](jb)
            row.append(e)
        assign.append(row)
        if pos % 32 == 31:  # quadrant sigmoid lands on ACT
            loads["scalar"] += 0.833 * qmax(pos // 32) + 460.0
    return assign, loads


ASSIGN, _LOADS = _mk_assignment()


class _TC(tile.TileContext):
    """TileContext variant for a walrus build that only supports ONE sem
    wait per instruction: split multi-wait instructions by hoisting the
    extra waits onto NOPs inserted just before them."""

    MAXW = 1

    def _split_waits_in_list(self, insts):
        out = []
        for inst in insts:
            si = inst.sync_info
            waits = list(si.on_wait) if si is not None else []
            if len(waits) > self.MAXW:
                rest, keep = waits[: -self.MAXW], waits[-self.MAXW :]
                for i in range(0, len(rest), self.MAXW):
                    nop = mybir.InstNoOp(
                        name=self.nc.get_next_instruction_name(),
                        engine=inst.engine,
                        bass_nofuse=True,
                        sync_info=bass_rust.SyncInfo(
                            on_wait=rest[i : i + self.MAXW], on_update=[]
                        ),
                    )
                    out.append(nop)
                inst.sync_info = bass_rust.SyncInfo(
                    on_wait=keep, on_update=list(si.on_update)
                )
            out.append(inst)
        return out

    def _lower_ordered_insts(self, ordered):
        for bb_name in list(ordered.keys()):
            ordered[bb_name] = self._split_waits_in_list(ordered[bb_name])
        return super()._lower_ordered_insts(ordered)

    def _drain_and_barrier(self, tick_clock, wait_clock):
        drain_inst = self.nc.sync.drain()
        wait_clock.add_sem_waits(
            drain_inst.ins, ScopedClock({None: tick_clock.global_clock})
        )
        si = drain_inst.ins.sync_info
        waits = list(si.on_wait) if si is not None else []
        if len(waits) > self.MAXW:
            drain_inst.ins.sync_info = bass_rust.SyncInfo(
                on_wait=waits[: self.MAXW], on_update=list(si.on_update)
            )
            rest = waits[self.MAXW :]
            for i in range(0, len(rest), self.MAXW):
                nop = self.nc.sync.nop(nofuse=True, hint=f"dw{i}")
                nop.ins.sync_info = bass_rust.SyncInfo(
                    on_wait=rest[i : i + self.MAXW], on_update=[]
                )
        self.nc.all_engine_barrier()
        assert self.sems is not None
        popped = self.nc._tile_sem_poison_stack.pop()
        assert popped is self._sem_poison
        self.nc.clear_and_free_semaphores(list(self.sems.allocated().values()))
        self.nc.all_engine_barrier()


def _fetch_xt(nc, wpool, xt):
    NT = N + R
    xts = []
    xt_q = [nc.sync, nc.gpsimd]
    for c in range(2):
        t = wpool.tile([128, NT], BF16, name=f"xt_{c}", tag=f"xt_{c}")
        xt_q[c].dma_start(t[:, :], xt[c * 128 : (c + 1) * 128, :])
        xts.append(t)
    return xts


def _drain(nc, eng, dst, src, bias_col, func):
    """psum -> sbuf drain on the chosen engine."""
    if eng == "scalar":
        if func == "relu":
            nc.scalar.activation(dst, src, AF.Relu, bias=bias_col)
        elif func == "addbias":
            nc.scalar.activation(dst, src, AF.Identity, bias=bias_col)
        else:
            nc.scalar.copy(dst, src)
    else:
        if func == "relu":
            nc.vector.tensor_scalar(
                out=dst, in0=src, scalar1=bias_col, scalar2=0.0,
                op0=ALU.add, op1=ALU.max)
        elif func == "addbias":
            nc.vector.tensor_scalar(
                out=dst, in0=src, scalar1=bias_col, scalar2=None,
                op0=ALU.add)
        else:
            nc.vector.tensor_scalar(
                out=dst, in0=src, scalar1=0.0, scalar2=None, op0=ALU.add)


def _upstream(nc, wpool, ubanks, cbanks, wts, b1t, b2t, bep1t, rep, xts,
              split_parts=False):
    """h -> f -> (c into psum+sbuf, a).  Returns tiles dict for _pairwise.
    With split_parts, returns (tiles, emit_rest): the 256:512 token half is
    deferred (cold-start path for rep 0)."""
    NT = N + R
    cbk = cbanks[rep % 2]

    ubank_rr = [0]

    def next_ubank(tn):
        t = ubanks[ubank_rr[0] % len(ubanks)]
        ubank_rr[0] += 1
        return t[:, 0:tn]

    def mmslice(dst_tiles, src_tiles, wname, bias_tile, func, t0, tn, eng):
        for oc in range(2):
            ps = next_ubank(tn)
            for kc in range(2):
                nc.tensor.matmul(
                    ps,
                    lhsT=wts[(wname, kc)][:, oc * 128 : (oc + 1) * 128],
                    rhs=src_tiles[kc][:, t0 : t0 + tn],
                    start=(kc == 0), stop=(kc == 1),
                )
            bias_col = bias_tile[:, oc : oc + 1] if bias_tile is not None else None
            if eng == "vector2":
                deng = "vector"
            else:
                deng = eng if oc == 0 else ("vector" if eng == "scalar" else "scalar")
            _drain(nc, deng, dst_tiles[oc][:, t0 : t0 + tn], ps, bias_col, func)

    hts = [wpool.tile([128, NT], BF16, name=f"ht_{c}", tag=f"ht_{c}") for c in range(2)]
    fts = [wpool.tile([128, NT], BF16, name=f"ft_{c}", tag=f"ft_{c}") for c in range(2)]
    cts = [wpool.tile([128, N], BF16, name=f"ct_{c}", tag=f"ct_{c}") for c in range(2)]
    ats = [wpool.tile([128, R], F32, name=f"at_{c}", tag=f"at_{c}") for c in range(2)]

    def cslice(t0, tn):
        for oc in range(2):
            ps = cbk[oc][:, t0 : t0 + tn]
            for kc in range(2):
                nc.tensor.matmul(
                    ps,
                    lhsT=wts[("wbt", kc)][:, oc * 128 : (oc + 1) * 128],
                    rhs=fts[kc][:, t0 : t0 + tn],
                    start=(kc == 0), stop=(kc == 1),
                    skip_group_check=True,
                )
            _drain(nc, "vector" if oc == 0 else "scalar",
                   cts[oc][:, t0 : t0 + tn], ps, None, "copy")

    # own-token path first: h(512:640) -> f(512:640) -> ats
    # (drains on DVE: ACT is still loading its activation table)
    mmslice(hts, xts, "w1t", b1t, "relu", 512, 128, "vector2")
    mmslice(fts, hts, "w2t", b2t, "relu", 512, 128, "vector2")
    for oc in range(2):
        ps = next_ubank(R)
        for kc in range(2):
            nc.tensor.matmul(
                ps,
                lhsT=wts[("wat", kc)][:, oc * 128 : (oc + 1) * 128],
                rhs=fts[kc][:, 512 : 512 + R],
                start=(kc == 0), stop=(kc == 1),
            )
        _drain(nc, "vector",
               ats[oc][:, :], ps, bep1t[:, oc : oc + 1], "addbias")

    mmslice(hts, xts, "w1t", b1t, "relu", 0, 256, "scalar")
    mmslice(fts, hts, "w2t", b2t, "relu", 0, 256, "vector")
    cslice(0, 256)

    tiles = {"hts": hts, "fts": fts, "cts": cts, "ats": ats}

    def emit_rest():
        mmslice(hts, xts, "w1t", b1t, "relu", 256, 256, "scalar")
        mmslice(fts, hts, "w2t", b2t, "relu", 256, 256, "vector")
        cslice(256, 256)

    if split_parts:
        return tiles, emit_rest
    emit_rest()
    return tiles


def _pairwise(nc, epool, scpool, cbanks, sbank, zlhs, zrhs, y, wslt, bep2t,
              rep, tiles, emit_rest, hook85):
    """The 256 row MMs + e-gens; emit_rest (cold start) fires at pos 8,
    hook85 (next rep's upstream+xt prefetch) fires at pos 85."""
    cbk = cbanks[rep % 2]
    sbk = sbank[rep % 2]
    cts, ats = tiles["cts"], tiles["ats"]

    sc = scpool.tile([128, N], F32, name="sc", tag="sc")
    # quadrant 0 starts with ascending tiny rows -> needs an explicit
    # zero-init; quadrants 1-3 start with their max-extent row, whose
    # first MM carries start=True and zero-fills the whole region.
    nc.tensor.matmul(
        sbk[0:32, 0 : qmax(0)],
        lhsT=zlhs[:, 0:32],
        rhs=zrhs[:, 0 : qmax(0)],
        start=True, stop=False,
        tile_position=(0, 0),
        skip_group_check=True,
    )

    for pos in range(R):
        if pos == 8 and emit_rest is not None:
            emit_rest()
        if pos == 85 and hook85 is not None:
            hook85()
        k = ORDER[pos]
        u, m = pos // 32, pos % 32
        jb = jbx(k)
        for c in range(2):
            eng = ASSIGN[pos][c]
            e = epool.tile([128, N], BF16, name=f"e{c}", tag=f"e{c}", bufs=10)
            acol = ats[c][:, k : k + 1]
            if eng == "vector":
                nc.vector.tensor_scalar(
                    out=e[:, 0:jb], in0=cts[c][:, 0:jb], scalar1=acol,
                    scalar2=0.0, op0=ALU.add, op1=ALU.max)
            elif eng == "gpsimd":
                nc.gpsimd.tensor_scalar(
                    out=e[:, 0:jb], in0=cts[c][:, 0:jb], scalar1=acol,
                    scalar2=0.0, op0=ALU.add, op1=ALU.max)
            else:
                nc.scalar.activation(e[:, 0:jb], cbk[c][:, 0:jb], AF.Relu,
                                     bias=acol)
            nc.tensor.matmul(
                sbk[32 * u : 32 * u + 32, 0:jb],
                lhsT=wslt[:, 63 * c + 31 - m : 63 * c + 63 - m],
                rhs=e[:, 0:jb],
                start=(u > 0 and m == 0 and c == 0),
                stop=(m == 31 and c == 1),
                tile_position=(0, 32 * u),
                skip_group_check=True,
            )

    # single end-of-rep drain: one sigmoid + one contiguous DMA
    nc.scalar.activation(sc[:, :], sbk[:, :], AF.Sigmoid,
                         bias=bep2t[:, 0:1])
    nc.sync.dma_start(y[:, :], sc[:, :])


def build_nc(reps: int = 1) -> bass.Bass:
    nc = bass.Bass("TRN2", target_bir_lowering=False, debug=False)

    NT = N + R  # 640 token columns: 512 shared j-tokens + 128 own i-tokens

    xt = nc.dram_tensor("xt", [H, NT], BF16, kind="ExternalInput")
    # packed weights: [128, 512] each, chunk kc at cols [256*kc, 256*kc+256)
    wp = {nm: nc.dram_tensor(f"{nm}p", [128, 2 * H], BF16, kind="ExternalInput")
          for nm in ("w1t", "w2t", "wat", "wbt")}
    # packed small f32 constants: b1(2) b2(2) bep1(2) bep2(1) = 7 cols
    bpk = nc.dram_tensor("bpk", [128, 7], F32, kind="ExternalInput")
    wsl = nc.dram_tensor("wsl", [128, 126], BF16, kind="ExternalInput")
    y = nc.dram_tensor("y", [R, N], F32, kind="ExternalOutput")

    with _TC(nc) as tc:
        with tc.tile_pool(name="const", bufs=1) as cpool, \
             tc.tile_pool(name="work", bufs=3) as wpool, \
             tc.tile_pool(name="epool", bufs=3) as epool, \
             tc.tile_pool(name="scpool", bufs=4) as scpool:

            # rep-0 inputs first so the own-token path starts ASAP
            xts = _fetch_xt(nc, wpool, xt)

            # ---- constants (loaded once, reused across reps) ----
            wts = {}
            qengs = {"w1t": nc.sync, "w2t": nc.gpsimd,
                     "wat": nc.gpsimd, "wbt": nc.gpsimd}
            for nm in ("w1t", "w2t", "wat", "wbt"):
                t = cpool.tile([128, 2 * H], BF16, name=f"{nm}p")
                qengs[nm].dma_start(t[:, :], wp[nm][:, :])
                for c in range(2):
                    wts[(nm, c)] = t[:, c * H : (c + 1) * H]
            wslt = cpool.tile([128, 126], BF16, name="wslt")
            nc.sync.dma_start(wslt[:, :], wsl[:, :])
            bpt = cpool.tile([128, 7], F32, name="bpt")
            nc.scalar.dma_start(bpt[:, :], bpk[:, :])
            # warm the ACT table (sigmoid set also covers relu/identity/copy)
            actwarm0 = cpool.tile([128, 1], F32, name="actwarm0")
            nc.scalar.activation(actwarm0[:, :], bpt[:, 0:1], AF.Sigmoid)
            b1t = bpt[:, 0:2]
            b2t = bpt[:, 2:4]
            bep1t = bpt[:, 4:6]
            bep2t = bpt[:, 6:7]

            ppp = tc.alloc_tile_pool(name="psum_pair", bufs=1, space="PSUM")
            ubanks = [ppp.tile([128, N], F32, name=f"u{i}") for i in range(2)]
            cbanks = [[ppp.tile([128, N], F32, name=f"c{p}{c}") for c in range(2)]
                      for p in range(2)]
            sbank = [ppp.tile([128, N], F32, name=f"s{p}") for p in range(2)]

            zlhs = cpool.tile([128, 128], BF16, name="zlhs")
            nc.gpsimd.memset(zlhs[:, :], 0.0)
            zrhs = cpool.tile([128, N], BF16, name="zrhs")
            nc.gpsimd.memset(zrhs[:, :], 0.0)
            # score banks must be finite everywhere once (the end sigmoid
            # reads columns beyond each quadrant extent); all other banks
            # are start=True-written before any read.
            for t in sbank:
                nc.vector.memset(t[:, :], 0.0)
            tiles, emit_rest = _upstream(
                nc, wpool, ubanks, cbanks, wts, b1t, b2t, bep1t, 0, xts,
                split_parts=True)
            state = {"tiles": tiles, "emit_rest": emit_rest}

            def mk_hook(nxt_rep):
                if nxt_rep >= reps:
                    return None

                def hook():
                    nxts = _fetch_xt(nc, wpool, xt)
                    state["tiles"] = _upstream(
                        nc, wpool, ubanks, cbanks, wts, b1t, b2t, bep1t,
                        nxt_rep, nxts)
                    state["emit_rest"] = None

                return hook

            for rep in range(reps):
                _pairwise(nc, epool, scpool, cbanks, sbank, zlhs, zrhs, y,
                          wslt, bep2t, rep, state["tiles"],
                          state["emit_rest"], mk_hook(rep + 1))

            ppp.release()

    return nc


_NC_CACHE = {}


def _get_nc(reps: int = 1):
    if reps not in _NC_CACHE:
        _NC_CACHE[reps] = build_nc(reps)
    return _NC_CACHE[reps]


def make_in_maps(step_sequence, step_mask, W_gc1, b_gc1, W_gc2, b_gc2,
                 W_ep1, b_ep1, w_ep2, b_ep2):
    x = np.ascontiguousarray(np.asarray(step_sequence, dtype=np.float32))
    W_gc1 = np.asarray(W_gc1, np.float32)
    W_gc2 = np.asarray(W_gc2, np.float32)
    W_ep1 = np.asarray(W_ep1, np.float32)
    b_gc1 = np.asarray(b_gc1, np.float32)
    b_gc2 = np.asarray(b_gc2, np.float32)
    b_ep1 = np.asarray(b_ep1, np.float32)
    w_ep2 = np.asarray(w_ep2, np.float32)
    b_ep2v = np.float32(np.asarray(b_ep2))

    bf16 = ml_dtypes.bfloat16

    def pack_w(w):  # [H, H] -> [128, 512] (chunk kc at cols 256kc..)
        wt = np.ascontiguousarray(w.T)
        return np.ascontiguousarray(
            np.concatenate([wt[0:128, :], wt[128:256, :]], axis=1)
        ).astype(bf16)

    w1p = pack_w(W_gc1)
    w2p = pack_w(W_gc2)
    wap = pack_w(W_ep1[:, :H])
    wbp = pack_w(W_ep1[:, H:])
    bpk = np.zeros((128, 7), np.float32)
    bpk[:, 0:2] = b_gc1.reshape(2, 128).T
    bpk[:, 2:4] = b_gc2.reshape(2, 128).T
    bpk[:, 4:6] = b_ep1.reshape(2, 128).T
    bpk[:, 6] = b_ep2v
    wep2m = np.ascontiguousarray(w_ep2.reshape(2, 128).T)
    wslm = np.zeros((128, 126), np.float32)
    wslm[:, 31] = wep2m[:, 0]
    wslm[:, 63 + 31] = wep2m[:, 1]
    wslm = wslm.astype(bf16)

    in_maps = []
    for d in range(NCORES):
        b, ph = divmod(d, 4)
        my_i = np.arange(ph, N, 4)
        xT = x[b].T  # [H, N]
        xTmy = np.ascontiguousarray(x[b][my_i].T)  # [H, R]
        xt640 = np.ascontiguousarray(
            np.concatenate([xT, xTmy], axis=1)).astype(bf16)
        in_maps.append({
            "xt": xt640, "w1tp": w1p, "w2tp": w2p, "watp": wap, "wbtp": wbp,
            "bpk": bpk, "wsl": wslm,
        })
    return in_maps


_MASK_CACHE = {}


def _tril_mask():
    if "m" not in _MASK_CACHE:
        _MASK_CACHE["m"] = np.tril(np.ones((N, N), np.float32), k=-1)
    return _MASK_CACHE["m"]


POS_OF_ROW = [0] * R
for _p, _k in enumerate(ORDER):
    POS_OF_ROW[_k] = _p


def gather_output(results):
    out = np.zeros((B, N, N), np.float32)
    for d in range(NCORES):
        b, ph = divmod(d, 4)
        dev = results[d]["y"]  # [R, N]
        for k in range(R):
            jb = jbx(k)
            out[b, 4 * k + ph, :jb] = dev[POS_OF_ROW[k], :jb]
    out *= _tril_mask()[None, :, :]
    return out


def kernel(**inputs) -> np.ndarray:
    nc = _get_nc()
    in_maps = make_in_maps(**inputs)
    res = run_bass_kernel_spmd(nc, in_maps, core_ids=list(range(NCORES)))
    return gather_output(res.results)


# revision 4
# speedup vs baseline: 1.5234x; 1.5234x over previous
"""Trainium2 Bass kernel for CausalGraphNetwork — v2.

Computes, for x = step_sequence [B=2, N=512, H=256]:
    h  = relu(x @ W_gc1.T + b_gc1)
    f  = relu(h @ W_gc2.T + b_gc2)
    a  = f @ Wa.T + b_ep1    (Wa = W_ep1[:, :H])
    c  = f @ Wb.T            (Wb = W_ep1[:, H:])
    e[b,i,j,:] = relu(a[b,i,:] + c[b,j,:])
    scores = sigmoid(e @ w_ep2 + b_ep2) * strict_lower_mask

Strategy (8 NeuronCores, SPMD single program):
  - Core d owns batch d//4, rows i = 4k + (d%4), k = 0..127 (interleaved
    so causal work is identical on every core).
  - Row k's score row is accumulated at one PSUM partition of ONE score
    bank via a sliding one-hot lhsT: lhsT = wsl[:, 31-m:63-m] puts w at
    output row m of the 32-row column-tile quadrant (tile_position
    (0,32u)); all other rows accumulate +0.  128 rows -> one [128,512]
    bank, drained by 4 quadrant sigmoids + 4 contiguous DMAs.
  - e-gen (e = relu(c_j + a_k), per-partition bias) is split across
    DVE/Pool (tensor_scalar from SBUF c) and ACT (activation from a
    PSUM-resident copy of c, fused bias+relu), greedy-balanced.
  - Row processing order: k = 0..23 ascending (small causal extents,
    only c[:, :256] needed) then k = 127..24 descending, so the tail
    rows are cheap and quadrant drains overlap remaining compute.
  - Causality: row k computes jbx(k) = round8(4k+4) columns; the host
    applies the exact strict-lower mask after gathering.
"""

import ml_dtypes
import numpy as np

import bass_rust
import concourse.bass as bass
import concourse.mybir as mybir
import concourse.tile as tile
from concourse.bass_utils import run_bass_kernel_spmd
from concourse.vector_clock import ScopedClock

B, N, H = 2, 512, 256
NCORES = 8
R = 128  # rows per core
F32 = mybir.dt.float32
BF16 = mybir.dt.bfloat16
AF = mybir.ActivationFunctionType
ALU = mybir.AluOpType


def jbx(k: int) -> int:
    """Per-row compute extent (even, = 4k+4, capped at N)."""
    return min(N, 4 * k + 4)


# processing order: position -> row k
ORDER = list(range(24)) + list(range(127, 23, -1))
assert sorted(ORDER) == list(range(R))


def qmax(u: int) -> int:
    """Max compute extent among rows at positions 32u..32u+31."""
    return max(jbx(ORDER[p]) for p in range(32 * u, 32 * u + 32))


# ---- static engine assignment for the 256 e-gen chunk ops ----
# per-column ns cost model per engine (overridable for tuning experiments)
import os as _os

_CST = [float(v) for v in _os.environ.get(
    "EGEN_COSTS", "0.52,2.2,0.833").split(",")]


def _mk_assignment():
    def dve_cost(jb):
        return _CST[0] * jb + 30.0

    def pool_cost(jb):
        return _CST[1] * jb + 40.0

    def act_cost(jb):
        return _CST[2] * jb + 460.0

    loads = {"vector": 800.0, "gpsimd": 0.0, "scalar": 1800.0}
    costs = {"vector": dve_cost, "gpsimd": pool_cost, "scalar": act_cost}
    assign = []
    for pos in range(R):
        jb = jbx(ORDER[pos])
        row = []
        for c in range(2):
            opts = list(loads)
            if pos % 32 in (30, 31, 0, 1):  # keep ACT free around sigmoids
                opts = [o for o in opts if o != "scalar"]
            e = min(opts, key=lambda n: loads[n] + costs[n](jb))
            loads[e] += costs[e](jb)
            row.append(e)
        assign.append(row)
        if pos % 32 == 31:  # quadrant sigmoid lands on ACT
            loads["scalar"] += 0.833 * qmax(pos // 32) + 460.0
    return assign, loads


ASSIGN, _LOADS = _mk_assignment()


class _TC(tile.TileContext):
    """TileContext variant for a walrus build that only supports ONE sem
    wait per instruction: split multi-wait instructions by hoisting the
    extra waits onto NOPs inserted just before them."""

    MAXW = 1

    def _split_waits_in_list(self, insts):
        out = []
        for inst in insts:
            si = inst.sync_info
            waits = list(si.on_wait) if si is not None else []
            if len(waits) > self.MAXW:
                rest, keep = waits[: -self.MAXW], waits[-self.MAXW :]
                for i in range(0, len(rest), self.MAXW):
                    nop = mybir.InstNoOp(
                        name=self.nc.get_next_instruction_name(),
                        engine=inst.engine,
                        bass_nofuse=True,
                        sync_info=bass_rust.SyncInfo(
                            on_wait=rest[i : i + self.MAXW], on_update=[]
                        ),
                    )
                    out.append(nop)
                inst.sync_info = bass_rust.SyncInfo(
                    on_wait=keep, on_update=list(si.on_update)
                )
            out.append(inst)
        return out

    def _lower_ordered_insts(self, ordered):
        for bb_name in list(ordered.keys()):
            ordered[bb_name] = self._split_waits_in_list(ordered[bb_name])
        return super()._lower_ordered_insts(ordered)

    def _drain_and_barrier(self, tick_clock, wait_clock):
        drain_inst = self.nc.sync.drain()
        wait_clock.add_sem_waits(
            drain_inst.ins, ScopedClock({None: tick_clock.global_clock})
        )
        si = drain_inst.ins.sync_info
        waits = list(si.on_wait) if si is not None else []
        if len(waits) > self.MAXW:
            drain_inst.ins.sync_info = bass_rust.SyncInfo(
                on_wait=waits[: self.MAXW], on_update=list(si.on_update)
            )
            rest = waits[self.MAXW :]
            for i in range(0, len(rest), self.MAXW):
                nop = self.nc.sync.nop(nofuse=True, hint=f"dw{i}")
                nop.ins.sync_info = bass_rust.SyncInfo(
                    on_wait=rest[i : i + self.MAXW], on_update=[]
                )
        self.nc.all_engine_barrier()
        assert self.sems is not None
        popped = self.nc._tile_sem_poison_stack.pop()
        assert popped is self._sem_poison
        self.nc.clear_and_free_semaphores(list(self.sems.allocated().values()))
        self.nc.all_engine_barrier()


def _fetch_xt(nc, wpool, xt):
    NT = N + R
    xts = []
    xt_q = [nc.sync, nc.gpsimd]
    for c in range(2):
        t = wpool.tile([128, NT], BF16, name=f"xt_{c}", tag=f"xt_{c}")
        xt_q[c].dma_start(t[:, :], xt[c * 128 : (c + 1) * 128, :])
        xts.append(t)
    return xts


def _drain(nc, eng, dst, src, bias_col, func):
    """psum -> sbuf drain on the chosen engine."""
    if eng == "scalar":
        if func == "relu":
            nc.scalar.activation(dst, src, AF.Relu, bias=bias_col)
        elif func == "addbias":
            nc.scalar.activation(dst, src, AF.Identity, bias=bias_col)
        else:
            nc.scalar.copy(dst, src)
    else:
        if func == "relu":
            nc.vector.tensor_scalar(
                out=dst, in0=src, scalar1=bias_col, scalar2=0.0,
                op0=ALU.add, op1=ALU.max)
        elif func == "addbias":
            nc.vector.tensor_scalar(
                out=dst, in0=src, scalar1=bias_col, scalar2=None,
                op0=ALU.add)
        else:
            nc.vector.tensor_scalar(
                out=dst, in0=src, scalar1=0.0, scalar2=None, op0=ALU.add)


def _upstream(nc, wpool, ubanks, cbanks, wts, b1t, b2t, bep1t, rep, xts,
              split_parts=False):
    """h -> f -> (c into psum+sbuf, a).  Returns tiles dict for _pairwise.
    With split_parts, returns (tiles, emit_rest): the 256:512 token half is
    deferred (cold-start path for rep 0)."""
    NT = N + R
    cbk = cbanks[rep % 2]

    ubank_rr = [0]

    def next_ubank(tn):
        t = ubanks[ubank_rr[0] % len(ubanks)]
        ubank_rr[0] += 1
        return t[:, 0:tn]

    def mmslice(dst_tiles, src_tiles, wname, bias_tile, func, t0, tn, eng):
        for oc in range(2):
            ps = next_ubank(tn)
            for kc in range(2):
                nc.tensor.matmul(
                    ps,
                    lhsT=wts[(wname, kc)][:, oc * 128 : (oc + 1) * 128],
                    rhs=src_tiles[kc][:, t0 : t0 + tn],
                    start=(kc == 0), stop=(kc == 1),
                )
            bias_col = bias_tile[:, oc : oc + 1] if bias_tile is not None else None
            if eng == "vector2":
                deng = "vector"
            else:
                deng = eng if oc == 0 else ("vector" if eng == "scalar" else "scalar")
            _drain(nc, deng, dst_tiles[oc][:, t0 : t0 + tn], ps, bias_col, func)

    hts = [wpool.tile([128, NT], BF16, name=f"ht_{c}", tag=f"ht_{c}") for c in range(2)]
    fts = [wpool.tile([128, NT], BF16, name=f"ft_{c}", tag=f"ft_{c}") for c in range(2)]
    cts = [wpool.tile([128, N], BF16, name=f"ct_{c}", tag=f"ct_{c}") for c in range(2)]
    ats = [wpool.tile([128, R], F32, name=f"at_{c}", tag=f"at_{c}") for c in range(2)]

    def cslice(t0, tn):
        for oc in range(2):
            ps = cbk[oc][:, t0 : t0 + tn]
            for kc in range(2):
                nc.tensor.matmul(
                    ps,
                    lhsT=wts[("wbt", kc)][:, oc * 128 : (oc + 1) * 128],
                    rhs=fts[kc][:, t0 : t0 + tn],
                    start=(kc == 0), stop=(kc == 1),
                    skip_group_check=True,
                )
            _drain(nc, "vector" if oc == 0 else "scalar",
                   cts[oc][:, t0 : t0 + tn], ps, None, "copy")

    # own-token path first: h(512:640) -> f(512:640) -> ats
    # (drains on DVE: ACT is still loading its activation table)
    mmslice(hts, xts, "w1t", b1t, "relu", 512, 128, "vector2")
    mmslice(fts, hts, "w2t", b2t, "relu", 512, 128, "vector2")
    for oc in range(2):
        ps = next_ubank(R)
        for kc in range(2):
            nc.tensor.matmul(
                ps,
                lhsT=wts[("wat", kc)][:, oc * 128 : (oc + 1) * 128],
                rhs=fts[kc][:, 512 : 512 + R],
                start=(kc == 0), stop=(kc == 1),
            )
        _drain(nc, "vector",
               ats[oc][:, :], ps, bep1t[:, oc : oc + 1], "addbias")

    mmslice(hts, xts, "w1t", b1t, "relu", 0, 256, "scalar")
    mmslice(fts, hts, "w2t", b2t, "relu", 0, 256, "vector")
    cslice(0, 256)

    tiles = {"hts": hts, "fts": fts, "cts": cts, "ats": ats}

    def emit_rest():
        mmslice(hts, xts, "w1t", b1t, "relu", 256, 256, "scalar")
        mmslice(fts, hts, "w2t", b2t, "relu", 256, 256, "vector")
        cslice(256, 256)

    if split_parts:
        return tiles, emit_rest
    emit_rest()
    return tiles


def _pairwise(nc, epool, scpool, cbanks, sbank, zlhs, zrhs, y, wslt, bep2t,
              rep, tiles, emit_rest, hook85):
    """The 256 row MMs + e-gens; emit_rest (cold start) fires at pos 8,
    hook85 (next rep's upstream+xt prefetch) fires at pos 85."""
    cbk = cbanks[rep % 2]
    sbk = sbank[rep % 2]
    cts, ats = tiles["cts"], tiles["ats"]

    sc = scpool.tile([128, N], F32, name="sc", tag="sc")
    # quadrant 0 starts with ascending tiny rows -> needs an explicit
    # zero-init; quadrants 1-3 start with their max-extent row, whose
    # first MM carries start=True and zero-fills the whole region.
    nc.tensor.matmul(
        sbk[0:32, 0 : qmax(0)],
        lhsT=zlhs[:, 0:32],
        rhs=zrhs[:, 0 : qmax(0)],
        start=True, stop=False,
        tile_position=(0, 0),
        skip_group_check=True,
    )

    for pos in range(R):
        if pos == 8 and emit_rest is not None:
            emit_rest()
        if pos == 85 and hook85 is not None:
            hook85()
        k = ORDER[pos]
        u, m = pos // 32, pos % 32
        jb = jbx(k)
        for c in range(2):
            eng = ASSIGN[pos][c]
            e = epool.tile([128, N], BF16, name=f"e{c}", tag=f"e{c}", bufs=10)
            acol = ats[c][:, k : k + 1]
            if eng == "vector":
                nc.vector.tensor_scalar(
                    out=e[:, 0:jb], in0=cts[c][:, 0:jb], scalar1=acol,
                    scalar2=0.0, op0=ALU.add, op1=ALU.max)
            elif eng == "gpsimd":
                nc.gpsimd.tensor_scalar(
                    out=e[:, 0:jb], in0=cts[c][:, 0:jb], scalar1=acol,
                    scalar2=0.0, op0=ALU.add, op1=ALU.max)
            else:
                nc.scalar.activation(e[:, 0:jb], cbk[c][:, 0:jb], AF.Relu,
                                     bias=acol)
            nc.tensor.matmul(
                sbk[32 * u : 32 * u + 32, 0:jb],
                lhsT=wslt[:, 63 * c + 31 - m : 63 * c + 63 - m],
                rhs=e[:, 0:jb],
                start=(u > 0 and m == 0 and c == 0),
                stop=(m == 31 and c == 1),
                tile_position=(0, 32 * u),
                skip_group_check=True,
            )

    # single end-of-rep drain: one sigmoid + one contiguous DMA
    nc.scalar.activation(sc[:, :], sbk[:, :], AF.Sigmoid,
                         bias=bep2t[:, 0:1])
    nc.sync.dma_start(y[:, :], sc[:, :])


def build_nc(reps: int = 1) -> bass.Bass:
    nc = bass.Bass("TRN2", target_bir_lowering=False, debug=False)

    NT = N + R  # 640 token columns: 512 shared j-tokens + 128 own i-tokens

    xt = nc.dram_tensor("xt", [H, NT], BF16, kind="ExternalInput")
    # packed weights: [128, 512] each, chunk kc at cols [256*kc, 256*kc+256)
    wp = {nm: nc.dram_tensor(f"{nm}p", [128, 2 * H], BF16, kind="ExternalInput")
          for nm in ("w1t", "w2t", "wat", "wbt")}
    # packed small f32 constants: b1(2) b2(2) bep1(2) bep2(1) = 7 cols
    bpk = nc.dram_tensor("bpk", [128, 7], F32, kind="ExternalInput")
    wsl = nc.dram_tensor("wsl", [128, 126], BF16, kind="ExternalInput")
    y = nc.dram_tensor("y", [R, N], F32, kind="ExternalOutput")

    with _TC(nc) as tc:
        with tc.tile_pool(name="const", bufs=1) as cpool, \
             tc.tile_pool(name="work", bufs=3) as wpool, \
             tc.tile_pool(name="epool", bufs=3) as epool, \
             tc.tile_pool(name="scpool", bufs=4) as scpool:

            # rep-0 inputs first so the own-token path starts ASAP
            xts = _fetch_xt(nc, wpool, xt)

            # ---- constants (loaded once, reused across reps) ----
            wts = {}
            qengs = {"w1t": nc.sync, "w2t": nc.gpsimd,
                     "wat": nc.gpsimd, "wbt": nc.gpsimd}
            for nm in ("w1t", "w2t", "wat", "wbt"):
                t = cpool.tile([128, 2 * H], BF16, name=f"{nm}p")
                qengs[nm].dma_start(t[:, :], wp[nm][:, :])
                for c in range(2):
                    wts[(nm, c)] = t[:, c * H : (c + 1) * H]
            wslt = cpool.tile([128, 126], BF16, name="wslt")
            nc.sync.dma_start(wslt[:, :], wsl[:, :])
            bpt = cpool.tile([128, 7], F32, name="bpt")
            nc.scalar.dma_start(bpt[:, :], bpk[:, :])
            # warm the ACT table (sigmoid set also covers relu/identity/copy)
            actwarm0 = cpool.tile([128, 1], F32, name="actwarm0")
            nc.scalar.activation(actwarm0[:, :], bpt[:, 0:1], AF.Sigmoid)
            b1t = bpt[:, 0:2]
            b2t = bpt[:, 2:4]
            bep1t = bpt[:, 4:6]
            bep2t = bpt[:, 6:7]

            ppp = tc.alloc_tile_pool(name="psum_pair", bufs=1, space="PSUM")
            ubanks = [ppp.tile([128, N], F32, name=f"u{i}") for i in range(2)]
            cbanks = [[ppp.tile([128, N], F32, name=f"c{p}{c}") for c in range(2)]
                      for p in range(2)]
            sbank = [ppp.tile([128, N], F32, name=f"s{p}") for p in range(2)]

            zlhs = cpool.tile([128, 128], BF16, name="zlhs")
            nc.gpsimd.memset(zlhs[:, :], 0.0)
            zrhs = cpool.tile([128, N], BF16, name="zrhs")
            nc.gpsimd.memset(zrhs[:, :], 0.0)
            # score banks must be finite everywhere once (the end sigmoid
            # reads columns beyond each quadrant extent); all other banks
            # are start=True-written before any read.
            for t in sbank:
                nc.vector.memset(t[:, :], 0.0)
            tiles, emit_rest = _upstream(
                nc, wpool, ubanks, cbanks, wts, b1t, b2t, bep1t, 0, xts,
                split_parts=True)
            state = {"tiles": tiles, "emit_rest": emit_rest}

            def mk_hook(nxt_rep):
                if nxt_rep >= reps:
                    return None

                def hook():
                    nxts = _fetch_xt(nc, wpool, xt)
                    state["tiles"] = _upstream(
                        nc, wpool, ubanks, cbanks, wts, b1t, b2t, bep1t,
                        nxt_rep, nxts)
                    state["emit_rest"] = None

                return hook

            for rep in range(reps):
                _pairwise(nc, epool, scpool, cbanks, sbank, zlhs, zrhs, y,
                          wslt, bep2t, rep, state["tiles"],
                          state["emit_rest"], mk_hook(rep + 1))

            ppp.release()

    return nc


_NC_CACHE = {}


def _get_nc(reps: int = 1):
    if reps not in _NC_CACHE:
        _NC_CACHE[reps] = build_nc(reps)
    return _NC_CACHE[reps]


def make_in_maps(step_sequence, step_mask, W_gc1, b_gc1, W_gc2, b_gc2,
                 W_ep1, b_ep1, w_ep2, b_ep2):
    x = np.ascontiguousarray(np.asarray(step_sequence, dtype=np.float32))
    W_gc1 = np.asarray(W_gc1, np.float32)
    W_gc2 = np.asarray(W_gc2, np.float32)
    W_ep1 = np.asarray(W_ep1, np.float32)
    b_gc1 = np.asarray(b_gc1, np.float32)
    b_gc2 = np.asarray(b_gc2, np.float32)
    b_ep1 = np.asarray(b_ep1, np.float32)
    w_ep2 = np.asarray(w_ep2, np.float32)
    b_ep2v = np.float32(np.asarray(b_ep2))

    bf16 = ml_dtypes.bfloat16

    def pack_w(w):  # [H, H] -> [128, 512] (chunk kc at cols 256kc..)
        wt = np.ascontiguousarray(w.T)
        return np.ascontiguousarray(
            np.concatenate([wt[0:128, :], wt[128:256, :]], axis=1)
        ).astype(bf16)

    w1p = pack_w(W_gc1)
    w2p = pack_w(W_gc2)
    wap = pack_w(W_ep1[:, :H])
    wbp = pack_w(W_ep1[:, H:])
    bpk = np.zeros((128, 7), np.float32)
    bpk[:, 0:2] = b_gc1.reshape(2, 128).T
    bpk[:, 2:4] = b_gc2.reshape(2, 128).T
    bpk[:, 4:6] = b_ep1.reshape(2, 128).T
    bpk[:, 6] = b_ep2v
    wep2m = np.ascontiguousarray(w_ep2.reshape(2, 128).T)
    wslm = np.zeros((128, 126), np.float32)
    wslm[:, 31] = wep2m[:, 0]
    wslm[:, 63 + 31] = wep2m[:, 1]
    wslm = wslm.astype(bf16)

    in_maps = []
    for d in range(NCORES):
        b, ph = divmod(d, 4)
        my_i = np.arange(ph, N, 4)
        xT = x[b].T  # [H, N]
        xTmy = np.ascontiguousarray(x[b][my_i].T)  # [H, R]
        xt640 = np.ascontiguousarray(
            np.concatenate([xT, xTmy], axis=1)).astype(bf16)
        in_maps.append({
            "xt": xt640, "w1tp": w1p, "w2tp": w2p, "watp": wap, "wbtp": wbp,
            "bpk": bpk, "wsl": wslm,
        })
    return in_maps


_MASK_CACHE = {}


def _tril_mask():
    if "m" not in _MASK_CACHE:
        _MASK_CACHE["m"] = np.tril(np.ones((N, N), np.float32), k=-1)
    return _MASK_CACHE["m"]


POS_OF_ROW = [0] * R
for _p, _k in enumerate(ORDER):
    POS_OF_ROW[_k] = _p


def gather_output(results):
    out = np.zeros((B, N, N), np.float32)
    for d in range(NCORES):
        b, ph = divmod(d, 4)
        dev = results[d]["y"]  # [R, N]
        for k in range(R):
            jb = jbx(k)
            out[b, 4 * k + ph, :jb] = dev[POS_OF_ROW[k], :jb]
    out *= _tril_mask()[None, :, :]
    return out


def kernel(**inputs) -> np.ndarray:
    nc = _get_nc()
    in_maps = make_in_maps(**inputs)
    res = run_bass_kernel_spmd(nc, in_maps, core_ids=list(range(NCORES)))
    return gather_output(res.results)
